# revision 1
# baseline (speedup 1.0000x reference)
"""Trainium2 Bass kernel for nn_CNN_Nested (W2NER-style CNN scorer).

Math (reference):
  head = leaky(wr @ head_w.T + head_b); tail likewise           [B,N,D]
  scores1[b,(h,d),l,k] = sum_{x,y} head[b,l,h,x] U[h,d,x,y] tail[b,k,h,y]
  scores2[b,c,m,n] = h_aug@Wh.T (bcast n) + t_aug@Wt.T (bcast m) + size-emb term
  out = down_w @ (scores1+scores2) + down_b                     [B,OUT,N,N]

down_fc is linear => fold down_w into the constants on the host:
  U'[o,h,x,y] = sum_d down_w[o,h*HD+d] U[h,d,x,y]
  WhD = down_w @ Wh, WtD = down_w @ Wt               (tiny)
  E[o,m,n] = (size_emb @ (down_w@Ws).T)[clip(n-m)+15, o] + down_b[o]
Then per (b, o):
  G[o] = blockdiag(U'[o])^T @ tailT                  [(h,x)=200, N]
  out[b,o] = headT^T @ G[o] + A'[o,m] (x) 1 + 1 (x) B'[o,n] + E[o]
The broadcast adds ride along the group-B matmul: headT_B is augmented with
a ones row (-> B' via gB's B'-row) and the six A'T rows (-> A' via per-pair
indicator rows in gB).

All matmul operands are bf16 (PSUM accumulation stays f32); E is added in
bf16 by DVE during PSUM eviction. wr arrives host-pre-transposed. Constants
ship as two [128, *] bf16 blobs (blob1 = wrT+tail weights so the first
matmuls start as early as possible).

Sharding: 8 cores = B(4) x o-half(2x6). No collectives. Full inputs in,
full output out. Hardcoded B=4,N=256,H=768,D=200,NH=5,HD=40,OUT=12.
"""

import os
import numpy as np

B, N, H = 4, 256, 768
D, NH, HD, SZ, OUT = 200, 5, 40, 25, 12
N_POS = 30
OH = OUT // 2          # o's per core
NCORES = 8
GA, GB = 3 * HD, 2 * HD  # 120 / 80: d-rows in partition group A / B
GBX = GB + 7             # group-B rows + ones row + 6 A'T rows

_cache = {}
LAST_RESULT = None


def _layout(has_bias):
    """Column maps for the two constant blobs: name -> (blob, col0, ncols)."""
    cols = {}
    c = [0, 0]

    def add(blob, name, ncols):
        cols[name] = (blob, c[blob], ncols)
        c[blob] += ncols

    add(0, 'wrt', 6 * N)       # interleaved per-chunk [wrt_k|tw_k] segments
    add(0, 'tw_t', 6 * D)
    add(1, 'hw_t', 6 * D)
    add(1, 'ones', N)
    add(1, 'bd_a', OH * GA)
    add(1, 'bd_b', OH * GB)
    add(1, 'whdt_a', OH)
    add(1, 'whdt_b', OH)
    add(1, 'wtdt_a', OH)
    add(1, 'wtdt_b', OH)
    add(1, 'indp', 3 * 512)    # per-pair A' indicator rows [6, 512] x 3
    if has_bias:
        add(1, 'hb_a', 2)
        add(1, 'hb_b', 2)
        add(1, 'tb_a', 2)
        add(1, 'tb_b', 2)
    return cols, c


def _build_module(has_bias: bool):
    import concourse.bacc as bacc
    import concourse.mybir as mybir
    import concourse.tile as tile
    from concourse.bass import ts
    from contextlib import ExitStack

    dt = mybir.dt
    f32 = dt.float32
    bf = dt.bfloat16
    COLS, CN = _layout(has_bias)

    nc = bacc.Bacc("TRN2", target_bir_lowering=False, debug=False,
                   enable_asserts=False, enable_partition_id=False)

    b1_d = nc.dram_tensor("blob1", [128, CN[0]], bf, kind="ExternalInput").ap()
    b2_d = nc.dram_tensor("blob2", [128, CN[1]], bf, kind="ExternalInput").ap()
    e_d = nc.dram_tensor("e_pack", [128, OH * 512], bf,
                         kind="ExternalInput").ap()
    out_d = nc.dram_tensor("out", [OH, N, N], f32, kind="ExternalOutput").ap()

    with tile.TileContext(nc) as tc, ExitStack() as ctx:
        sb = ctx.enter_context(tc.tile_pool(name="sb", bufs=1))
        tmp_pool = ctx.enter_context(tc.tile_pool(name="tmp", bufs=2))
        pa_stack = ExitStack()
        pa = pa_stack.enter_context(tc.tile_pool(name="pa", bufs=2,
                                                 space="PSUM"))

        # PE warmup: keep TensorE busy during the input DMAs so the HAM
        # clock gate is fully open when real matmuls start.
        scratch = sb.tile([128, 512], bf, tag="warm", name="warm")
        nc.gpsimd.memset(scratch[:], 0.0)
        for _ in range(4):
            wps = pa.tile([128, 512], f32, tag="wps", name="wps", bufs=1)
            nc.tensor.matmul(wps[:], scratch[0:128, 0:128], scratch[:],
                             start=True, stop=True)

        # blob1 = six per-chunk [wrt_k|tw_k] segments (456 cols each),
        # loaded as chunk0 / chunks1-2 / chunks3-5 so work starts asap.
        SEG = N + D
        b10_s = sb.tile([128, SEG], bf, tag="b10", name="b10")
        nc.sync.dma_start(b10_s[:], b1_d[:, 0:SEG])
        b11_s = sb.tile([128, 2 * SEG], bf, tag="b11", name="b11")
        nc.scalar.dma_start(b11_s[:], b1_d[:, SEG:3 * SEG])
        b12_s = sb.tile([128, 3 * SEG], bf, tag="b12", name="b12")
        nc.sync.dma_start(b12_s[:], b1_d[:, 3 * SEG:6 * SEG])
        B2SPLIT = 6 * D + N  # hw_t + ones
        b2a_s = sb.tile([128, B2SPLIT], bf, tag="b2a", name="b2a")
        nc.scalar.dma_start(b2a_s[:], b2_d[:, 0:B2SPLIT])
        b2b_s = sb.tile([128, CN[1] - B2SPLIT], bf, tag="b2b", name="b2b")
        nc.sync.dma_start(b2b_s[:], b2_d[:, B2SPLIT:])
        e_s = sb.tile([128, OH * 512], bf, tag="es", name="es")
        nc.scalar.dma_start(e_s[:], e_d[:, :])

        def w(name, rows):
            blob, c0, cn = COLS[name]
            assert blob == 1
            if c0 < B2SPLIT:
                return b2a_s[0:rows, c0:c0 + cn]
            return b2b_s[0:rows, c0 - B2SPLIT:c0 - B2SPLIT + cn]

        def _seg(k):
            if k == 0:
                return b10_s, 0
            if k < 3:
                return b11_s, (k - 1) * SEG
            return b12_s, (k - 3) * SEG

        def wrT(k):
            t, c = _seg(k)
            return t[:, c:c + N]

        def tw_slice(k, off, sz):
            t, c = _seg(k)
            return t[:, c + N + off:c + N + off + sz]

        # ---- headT/tailT = leaky(w @ wr^T + b), [d, l] layout ---------------
        # group A rows d in [0,120); group B rows d in [120,200), then a ones
        # row at 80 and the six A'T rows at 81..86 (written later).
        headT_A = sb.tile([GA, N], bf, tag="hTA", name="hTA")
        headT_B = sb.tile([GBX, N], bf, tag="hTB", name="hTB")
        tailT_A = sb.tile([GA, N], bf, tag="tTA", name="tTA")
        tailT_B = sb.tile([GB + 1, N], bf, tag="tTB", name="tTB")

        def mlp(wsl, bname, off, sz, dst):
            ps = pa.tile([sz, N], f32, tag="pmlp", name="pmlp", bufs=4)
            for hk in range(6):
                nc.tensor.matmul(ps[:], wsl(hk, off, sz),
                                 wrT(hk), start=(hk == 0), stop=(hk == 5))
            if has_bias:
                tsc = tmp_pool.tile([sz, N], f32, tag="tsc", name="tsc")
                tln = tmp_pool.tile([sz, N], f32, tag="tln", name="tln")
                bias = w(bname, sz)
                nc.scalar.activation(tln[:], ps[:],
                                     mybir.ActivationFunctionType.Copy,
                                     bias=bias[:, 0:1])
                nc.scalar.activation(tsc[:], ps[:],
                                     mybir.ActivationFunctionType.Copy,
                                     bias=bias[:, 1:2], scale=0.01)
                nc.vector.tensor_max(dst, tln[:], tsc[:])
            else:
                tsc = tmp_pool.tile([sz, N], f32, tag="tsc", name="tsc")
                nc.scalar.activation(tsc[:], ps[:],
                                     mybir.ActivationFunctionType.Copy,
                                     scale=0.01)
                nc.vector.tensor_max(dst, ps[:], tsc[:])

        def hw_slice(k, off, sz):
            base = w('hw_t', 128)
            c = k * D + off
            return base[:, c:c + sz]

        mlp(tw_slice, 'tb_a', 0, GA, tailT_A[:])
        mlp(tw_slice, 'tb_b', GA, GB, tailT_B[0:GB, :])
        nc.sync.dma_start(tailT_B[GB:GB + 1, :], w('ones', 1))
        mlp(hw_slice, 'hb_a', 0, GA, headT_A[:])
        mlp(hw_slice, 'hb_b', GA, GB, headT_B[0:GB, :])
        nc.sync.dma_start(headT_B[GB:GB + 1, :], w('ones', 1))

        # ---- B'T / A'T projections [OH, N] ----------------------------------
        def proj(wa, wb, srcA, srcB, tagc):
            ps = pa.tile([OH, N], f32, tag="pap", name="pap", bufs=1)
            nc.tensor.matmul(ps[:], w(wa, GA), srcA[:], start=True, stop=False)
            nc.tensor.matmul(ps[:], w(wb, GB + 1), srcB[0:GB + 1, :],
                             start=False, stop=True)
            flat = sb.tile([OH, N], bf, tag=f"{tagc}f", name=f"{tagc}f")
            nc.scalar.copy(flat[:], ps[:])
            return flat

        Bpf = proj('wtdt_a', 'wtdt_b', tailT_A, tailT_B, "Bp")
        Apf = proj('whdt_a', 'whdt_b', headT_A, headT_B, "Ap")
        nc.sync.dma_start(headT_B[GB + 1:GBX, :], Apf[:, :])

        pa_stack.close()
        pg = ctx.enter_context(tc.tile_pool(name="pg", bufs=2,
                                            space="PSUM"))
        po = ctx.enter_context(tc.tile_pool(name="po", bufs=3, space="PSUM"))

        gAt, gBt = [], []

        def g_build(p):
            gA = sb.tile([GA, 512], bf, tag=f"gA{p}", name=f"gA{p}")
            gB = sb.tile([GBX, 512], bf, tag=f"gB{p}", name=f"gB{p}")
            for half in range(2):
                j = 2 * p + half
                psa = pg.tile([GA, N], f32, tag="psga", name="psga")
                nc.tensor.matmul(psa[:], w('bd_a', GA)[:, ts(j, GA)],
                                 tailT_A[:], start=True, stop=True)
                nc.scalar.copy(gA[:, ts(half, N)], psa[:])
                psb = pg.tile([GB, N], f32, tag="psgb", name="psgb")
                nc.tensor.matmul(psb[:], w('bd_b', GB)[:, ts(j, GB)],
                                 tailT_B[0:GB, :], start=True, stop=True)
                nc.vector.tensor_copy(gB[0:GB, ts(half, N)], psb[:])
            # row 80: [B'T[o0] | B'T[o1]]  (pairs with headT_B's ones row)
            nc.sync.dma_start(gB[GB:GB + 1, :], Bpf[2 * p:2 * p + 2, :])
            # rows 81..86: indicator rows (pair with headT_B's A'T rows)
            nc.gpsimd.dma_start(gB[GB + 1:GBX, :],
                                w('indp', OH)[:, ts(p, 512)])
            gAt.append(gA)
            gBt.append(gB)

        def out_bank(p):
            out_s = sb.tile([128, 1024], f32, tag=f"os{p}", name=f"os{p}")
            for lt in range(2):
                ob = po.tile([128, 512], f32, tag="ob", name="ob")
                nc.tensor.matmul(ob[:], headT_A[:, ts(lt, 128)], gAt[p][:],
                                 start=True, stop=False)
                nc.tensor.matmul(ob[:], headT_B[:, ts(lt, 128)], gBt[p][:],
                                 start=False, stop=True)
                nc.vector.tensor_add(out_s[:, ts(lt, 512)], ob[:],
                                     e_s[:, ts(2 * p + lt, 512)])
            # stores: per o, gather both lt blocks (3-dim APs for DMA balance)
            for half in range(2):
                dst = out_d[2 * p + half].rearrange("(t q) n -> q t n", q=128)
                srcap = out_s[:].rearrange("q (t o n) -> q t o n",
                                           t=2, o=2)[:, :, half]
                eng = nc.sync if half == 0 else nc.scalar
                eng.dma_start(dst, srcap)

        g_build(0)
        g_build(1)
        out_bank(0)
        g_build(2)
        out_bank(1)
        out_bank(2)

    nc.compile()
    return nc


def _get_module(has_bias: bool):
    key = ("mod", has_bias)
    if key not in _cache:
        _cache[key] = _build_module(has_bias)
    return _cache[key]


def _host_pack(head_w, head_b, tail_w, tail_b, U_mh, size_emb, W, down_w,
               down_b):
    """Fold down_w into the constants; build per-o-half bf16 blob layouts."""
    from ml_dtypes import bfloat16
    f64 = np.float64
    d1 = D + 1
    Wh, Wt, Ws = W[:, :d1], W[:, d1:2 * d1], W[:, 2 * d1:]
    WhD = (down_w.astype(f64) @ Wh.astype(f64)).astype(np.float32)   # [OUT,D+1]
    WtD = (down_w.astype(f64) @ Wt.astype(f64)).astype(np.float32)
    WsD = (down_w.astype(f64) @ Ws.astype(f64)).astype(np.float32)   # [OUT,SZ]
    ct = (size_emb.astype(f64) @ WsD.T.astype(f64)).astype(np.float32)
    dw_r = down_w.reshape(OUT, NH, HD)
    Up = np.einsum('ohd,hdxy->ohxy', dw_r.astype(f64),
                   U_mh.astype(f64)).astype(np.float32)              # [OUT,NH,HD,HD]

    idx = np.arange(N)
    span = np.clip(idx[None, :] - idx[:, None], -N_POS // 2,
                   N_POS // 2 - 1) + N_POS // 2
    E = ct[span].transpose(2, 0, 1) + down_b[:, None, None]          # [OUT,N,N]

    has_bias = bool(np.any(head_b) or np.any(tail_b))
    COLS, CN = _layout(has_bias)

    def pack_w(wmat):  # [D,H] -> [128, 6*200]
        return np.ascontiguousarray(
            wmat.T.reshape(6, 128, D).transpose(1, 0, 2).reshape(128, 6 * D))

    blobs_oh = []
    for oh in range(2):
        osl = slice(oh * OH, (oh + 1) * OH)
        blobs = [np.zeros((128, CN[0]), np.float32),
                 np.zeros((128, CN[1]), np.float32)]

        def put(name, arr):
            blob, c0, cn = COLS[name]
            r, cc = arr.shape
            assert cc == cn, (name, arr.shape)
            blobs[blob][0:r, c0:c0 + cn] = arr

        put('hw_t', pack_w(head_w))
        twp = pack_w(tail_w)
        SEG = N + D
        for k in range(6):
            blobs[0][:, k * SEG + N:(k + 1) * SEG] = twp[:, k * D:(k + 1) * D]
        UpS = Up[osl]
        bd_a = np.zeros((OH, GA, GA), np.float32)
        bd_b = np.zeros((OH, GB, GB), np.float32)
        for h in range(3):
            bd_a[:, h * HD:(h + 1) * HD, h * HD:(h + 1) * HD] = \
                UpS[:, h].transpose(0, 2, 1)
        for h in range(2):
            bd_b[:, h * HD:(h + 1) * HD, h * HD:(h + 1) * HD] = \
                UpS[:, 3 + h].transpose(0, 2, 1)
        put('bd_a', bd_a.transpose(1, 0, 2).reshape(GA, OH * GA))
        put('bd_b', bd_b.transpose(1, 0, 2).reshape(GB, OH * GB))
        put('whdt_a', WhD[osl, 0:GA].T)
        put('whdt_b', np.concatenate([WhD[osl, GA:D].T,
                                      WhD[osl, D:D + 1].T], axis=0))
        put('wtdt_a', WtD[osl, 0:GA].T)
        put('wtdt_b', np.concatenate([WtD[osl, GA:D].T,
                                      WtD[osl, D:D + 1].T], axis=0))
        indp = np.zeros((OH, 3 * 512), np.float32)
        for p in range(OH // 2):
            indp[2 * p, p * 512:p * 512 + N] = 1.0
            indp[2 * p + 1, p * 512 + N:p * 512 + 512] = 1.0
        put('indp', indp)
        put('ones', np.ones((1, N), np.float32))
        if has_bias:
            put('hb_a', np.stack([head_b[0:GA], 0.01 * head_b[0:GA]], axis=1))
            put('hb_b', np.stack([head_b[GA:D], 0.01 * head_b[GA:D]], axis=1))
            put('tb_a', np.stack([tail_b[0:GA], 0.01 * tail_b[0:GA]], axis=1))
            put('tb_b', np.stack([tail_b[GA:D], 0.01 * tail_b[GA:D]], axis=1))

        e_pack = np.zeros((128, OH * 512), np.float32)
        for p in range(OH // 2):
            for lt in range(2):
                o0 = oh * OH + 2 * p
                c0 = (2 * p + lt) * 512
                e_pack[:, c0:c0 + N] = E[o0, lt * 128:(lt + 1) * 128, :]
                e_pack[:, c0 + N:c0 + 512] = E[o0 + 1,
                                               lt * 128:(lt + 1) * 128, :]

        blobs_oh.append((blobs[0].astype(bfloat16), blobs[1].astype(bfloat16),
                         e_pack.astype(bfloat16)))
    return blobs_oh, has_bias, COLS


def _ensure_axon():
    """If a host-side jax.config pinned the cpu platform (e.g. to run the
    reference), switch back to the axon/neuron backend for the device run."""
    import jax
    try:
        if any(getattr(d, 'platform', '') == 'axon' for d in jax.devices()):
            return
    except Exception:
        pass
    try:
        import jax.extend
        jax.config.update('jax_platforms', 'axon')
        jax.extend.backend.clear_backends()
    except Exception:
        pass


def _make_in_maps(word_reps, blobs_oh):
    from ml_dtypes import bfloat16
    SEG = N + D
    wrt_b = []
    for b in range(B):
        wrt = word_reps[b].T.reshape(6, 128, N).transpose(1, 0, 2) \
            .reshape(128, 6 * N)
        wrt_b.append(wrt.astype(bfloat16))
    in_maps = []
    for core in range(NCORES):
        b, oh = core // 2, core % 2
        b1, b2, ep = blobs_oh[oh]
        b1 = b1.copy()
        for k in range(6):
            b1[:, k * SEG:k * SEG + N] = wrt_b[b][:, k * N:(k + 1) * N]
        in_maps.append(dict(blob1=b1, blob2=b2, e_pack=ep))
    return in_maps


def kernel(word_reps, cls_embeding=None, pieces_index=None, loss_mask=None,
           head_w=None, head_b=None, tail_w=None, tail_b=None, U_mh=None,
           size_emb=None, W=None, down_w=None, down_b=None, **_unused):
    global LAST_RESULT
    from concourse import bass_utils
    from ml_dtypes import bfloat16

    word_reps = np.asarray(word_reps, np.float32)
    args = [np.asarray(a, np.float32) for a in
            (head_w, head_b, tail_w, tail_b, U_mh, size_emb, W, down_w,
             down_b)]
    blobs_oh, has_bias, COLS = _host_pack(*args)

    nc = _get_module(has_bias)

    in_maps = _make_in_maps(word_reps, blobs_oh)
    _ensure_axon()

    trace = bool(os.environ.get("KERNEL_TRACE"))
    res = bass_utils.run_bass_kernel_spmd(nc, in_maps, list(range(NCORES)),
                                          trace=trace)
    LAST_RESULT = res

    out = np.empty((B, OUT, N, N), np.float32)
    for core in range(NCORES):
        b, oh = core // 2, core % 2
        out[b, oh * OH:(oh + 1) * OH] = res.results[core]["out"]
    return out



# revision 17
# speedup vs baseline: 1.1182x; 1.1182x over previous
"""Trainium2 Bass kernel for nn_CNN_Nested (W2NER-style CNN scorer).

Math (reference):
  head = leaky(wr @ head_w.T + head_b); tail likewise           [B,N,D]
  scores1[b,(h,d),l,k] = sum_{x,y} head[b,l,h,x] U[h,d,x,y] tail[b,k,h,y]
  scores2[b,c,m,n] = h_aug@Wh.T (bcast n) + t_aug@Wt.T (bcast m) + size-emb
  out = down_w @ (scores1+scores2) + down_b                     [B,OUT,N,N]

down_fc is linear => fold down_w into the constants on the host:
  U'[o,h,x,y] = sum_d down_w[o,h*HD+d] U[h,d,x,y]
  WhD = down_w @ Wh, WtD = down_w @ Wt               (tiny)
  E[o,m,n] = (size_emb @ (down_w@Ws).T)[clip(n-m)+15, o] + down_b[o]
Then per (b, o):
  G[o] = blockdiag(U'[o])^T @ tailT                  [(h,x)=200, N]
  out[b,o] = headT^T @ G[o] + A'[o,m] (x) 1 + 1 (x) B'[o,n] + E[o]
The broadcast adds ride along the group-B matmul: headT_B is augmented with
a ones row (-> B' via gB's B'-row) and the six A'T rows (-> A' via per-pair
indicator rows in gB).

Performance layout (v2):
  - blob1 interleaves per-128-contraction-chunk [wrT_k | tw_k | hw_k] so the
    MLP consumes chunks as they arrive (6 DMAs split over sync/scalar).
  - blob2 ships only live rectangles (no 128-row dead weight): bd blocks,
    proj weights, packed indicator rows.
  - E ships as fp8e4 (error ~0.06 absolute vs 0.19 budget).
  - output DRAM tensor is bf16 (host upcasts); halves store traffic.
  - evictions spread over scalar (Lrelu) / vector (gA, adds) / gpsimd (gB).

Sharding: 8 cores = B(4) x o-half(2x6). No collectives. Full inputs in,
full output out. Hardcoded B=4,N=256,H=768,D=200,NH=5,HD=40,OUT=12.
"""

import os
import numpy as np

B, N, H = 4, 256, 768
D, NH, HD, SZ, OUT = 200, 5, 40, 25, 12
N_POS = 30
OH = OUT // 2          # o's per core
NCORES = 8
GA, GB = 3 * HD, 2 * HD  # 120 / 80: d-rows in partition group A / B
GBX = GB + 7             # group-B rows + ones row + 6 A'T rows
NK = H // 128            # 6 contraction chunks
CHC = N + 2 * D          # 656 cols per blob1 chunk [wrt|tw|hw]

# blob2 column layout: rectA rows 0:120, rectB rows 0:GB+1, indp rows 0:6*3,
# ones row 0
RA_W = OH * GA + 2 * OH + 2   # bd_a | whdt_a | wtdt_a | hb_a | tb_a
RB_W = OH * GB + 2 * OH + 2
C_ONES = RA_W + RB_W + 512
C2 = C_ONES + 256

_cache = {}
LAST_RESULT = None


def _build_module(has_bias: bool):
    import concourse.bacc as bacc
    import concourse.mybir as mybir
    import concourse.tile as tile
    from concourse.bass import ts
    from contextlib import ExitStack

    dt = mybir.dt
    f32 = dt.float32
    bf = dt.bfloat16
    f8 = dt.float8e4

    nc = bacc.Bacc("TRN2", target_bir_lowering=False, debug=False,
                   enable_asserts=False, enable_partition_id=False)

    b1_d = nc.dram_tensor("blob1", [128, NK * CHC], bf,
                          kind="ExternalInput").ap()
    b2_d = nc.dram_tensor("blob2", [128, C2], bf, kind="ExternalInput").ap()
    e_d = nc.dram_tensor("e_pack", [128, OH * 512], f8,
                         kind="ExternalInput").ap()
    out_d = nc.dram_tensor("out", [OH, N, N], bf, kind="ExternalOutput").ap()

    with tile.TileContext(nc) as tc, ExitStack() as ctx:
        sb = ctx.enter_context(tc.tile_pool(name="sb", bufs=1))
        pa_stack = ExitStack()
        pa = pa_stack.enter_context(tc.tile_pool(name="pa", bufs=1,
                                                 space="PSUM"))

        # PE warmup: keep TensorE busy during the input DMAs so the HAM
        # clock gate opens before the real matmuls start.
        scratch = sb.tile([128, 512], bf, tag="warm", name="warm")
        nc.vector.memset(scratch[:], 0.0)
        for _ in range(3):
            wps = pa.tile([128, 512], f32, tag="wps", name="wps")
            nc.tensor.matmul(wps[:], scratch[0:128, 0:128], scratch[:],
                             start=True, stop=True)

        # ---- input DMAs --------------------------------------------------
        b1c = []
        for k in range(NK):
            t = sb.tile([128, CHC], bf, tag=f"b1c{k}", name=f"b1c{k}")
            eng = nc.sync if k % 2 == 0 else nc.scalar
            eng.dma_start(t[:], b1_d[:, k * CHC:(k + 1) * CHC])
            b1c.append(t)

        rectA = sb.tile([GA, RA_W], bf, tag="rectA", name="rectA")
        nc.gpsimd.dma_start(rectA[:], b2_d[0:GA, 0:RA_W])
        rectB = sb.tile([GB + 1, RB_W], bf, tag="rectB", name="rectB")
        nc.gpsimd.dma_start(rectB[:], b2_d[0:GB + 1, RA_W:RA_W + RB_W])
        e_s = sb.tile([128, OH * 512], f8, tag="es", name="es")
        nc.gpsimd.dma_start(e_s[:], e_d[:, :])

        def bd_a(j):
            return rectA[:, ts(j, GA)]

        def bd_b(j):
            return rectB[0:GB, ts(j, GB)]

        whdt_a = rectA[:, OH * GA:OH * GA + OH]
        wtdt_a = rectA[:, OH * GA + OH:OH * GA + 2 * OH]
        hb_a = rectA[:, OH * GA + 2 * OH:OH * GA + 2 * OH + 1]
        tb_a = rectA[:, OH * GA + 2 * OH + 1:OH * GA + 2 * OH + 2]
        whdt_b = rectB[:, OH * GB:OH * GB + OH]
        wtdt_b = rectB[:, OH * GB + OH:OH * GB + 2 * OH]
        hb_b = rectB[0:GB, OH * GB + 2 * OH:OH * GB + 2 * OH + 1]
        tb_b = rectB[0:GB, OH * GB + 2 * OH + 1:OH * GB + 2 * OH + 2]

        # ---- headT/tailT = leaky(w @ wr^T + b), [d, l] layout -------------
        headT_A = sb.tile([GA, N], bf, tag="hTA", name="hTA")
        headT_B = sb.tile([GBX, N], bf, tag="hTB", name="hTB")
        tailT_A = sb.tile([GA, N], bf, tag="tTA", name="tTA")
        tailT_B = sb.tile([GB + 1, N], bf, tag="tTB", name="tTB")
        nc.gpsimd.dma_start(tailT_B[GB:GB + 1, :],
                            b2_d[0:1, C_ONES:C_ONES + N])
        nc.gpsimd.dma_start(headT_B[GB:GB + 1, :],
                            b2_d[0:1, C_ONES:C_ONES + N])

        # interleaved accumulation groups must live in SEPARATE psum banks
        # (same-bank interleaved start/stop corrupts the accumulation)
        ps_tA = pa.tile([GA, N], f32, tag="ptA", name="ptA")
        ps_tB = pa.tile([GB, N], f32, tag="ptB", name="ptB")
        ps_hA = pa.tile([GA, N], f32, tag="phA", name="phA")
        ps_hB = pa.tile([GB, N], f32, tag="phB", name="phB")
        for k in range(NK):
            wrt = b1c[k][:, 0:N]
            st, sp = (k == 0), (k == NK - 1)
            nc.tensor.matmul(ps_tA[:], b1c[k][:, N:N + GA], wrt,
                             start=st, stop=sp)
            nc.tensor.matmul(ps_tB[:], b1c[k][:, N + GA:N + D], wrt,
                             start=st, stop=sp)
            nc.tensor.matmul(ps_hA[:], b1c[k][:, N + D:N + D + GA], wrt,
                             start=st, stop=sp)
            nc.tensor.matmul(ps_hB[:], b1c[k][:, N + D + GA:N + 2 * D], wrt,
                             start=st, stop=sp)

        lrelu = mybir.ActivationFunctionType.Lrelu

        def evict(dst, ps, bias):
            if has_bias:
                nc.scalar.activation(dst, ps, lrelu, bias=bias, alpha=0.01)
            else:
                nc.scalar.activation(dst, ps, lrelu, alpha=0.01)

        evict(tailT_A[:], ps_tA[:], tb_a)
        evict(tailT_B[0:GB, :], ps_tB[:], tb_b)
        evict(headT_A[:], ps_hA[:], hb_a)
        evict(headT_B[0:GB, :], ps_hB[:], hb_b)

        # ---- B'T / A'T projections (still in pa's banks) ------------------
        # both projections packed in one PSUM bank: cols 0:256 B', 256:512 A'
        ps_pj = pa.tile([OH, 512], f32, tag="ppj", name="ppj")
        pjf = sb.tile([OH, 512], bf, tag="pjf", name="pjf")

        def proj(col, wa, wb, srcA, srcB):
            nc.tensor.matmul(ps_pj[:, ts(col, N)], wa, srcA[:],
                             start=True, stop=False)
            nc.tensor.matmul(ps_pj[:, ts(col, N)], wb, srcB[0:GB + 1, :],
                             start=False, stop=True)

        proj(0, wtdt_a, wtdt_b, tailT_A, tailT_B)
        proj(1, whdt_a, whdt_b, headT_A, headT_B)
        nc.vector.tensor_copy(pjf[:], ps_pj[:])
        Apf = pjf[:, N:2 * N]
        nc.sync.dma_start(headT_B[GB + 1:GBX, :], Apf)

        pa_stack.close()
        pg = ctx.enter_context(tc.tile_pool(name="pg", bufs=2, space="PSUM"))
        po = ctx.enter_context(tc.tile_pool(name="po", bufs=3, space="PSUM"))
        gAt, gBt = [], []

        def g_build(p):
            gA = sb.tile([GA, 512], bf, tag=f"gA{p}", name=f"gA{p}")
            gB = sb.tile([GBX, 512], bf, tag=f"gB{p}", name=f"gB{p}")
            psa = pg.tile([GA, 512], f32, tag="psga", name="psga")
            psb = pg.tile([GB, 512], f32, tag="psgb", name="psgb")
            for half in range(2):
                j = 2 * p + half
                nc.tensor.matmul(psa[:, ts(half, N)], bd_a(j), tailT_A[:],
                                 start=True, stop=True)
                nc.tensor.matmul(psb[:, ts(half, N)], bd_b(j),
                                 tailT_B[0:GB, :], start=True, stop=True)
            nc.vector.tensor_copy(gA[:, :], psa[:])
            nc.scalar.copy(gB[0:GB, :], psb[:])
            # rows 81..86: indicator rows (pair with headT_B's A'T rows)
            nc.gpsimd.dma_start(gB[GB + 1:GBX, :],
                                b2_d[6 * p:6 * p + 6, RA_W + RB_W:
                                     RA_W + RB_W + 512])
            gAt.append(gA)
            gBt.append(gB)

        g_build(0)
        # row 80 of gB[p]: [B'T[o0] | B'T[o1]] (pairs with headT_B ones row)
        nc.sync.dma_start(gBt[0][GB:GB + 1, :], pjf[0:2, 0:N])
        g_build(1)
        nc.scalar.dma_start(gBt[1][GB:GB + 1, :], pjf[2:4, 0:N])

        def out_bank(p):
            out_s = sb.tile([128, 1024], bf, tag=f"os{p}", name=f"os{p}")
            for lt in range(2):
                ob = po.tile([128, 512], f32, tag="ob", name="ob")
                nc.tensor.matmul(ob[:], headT_A[:, ts(lt, 128)], gAt[p][:],
                                 start=True, stop=False)
                nc.tensor.matmul(ob[:], headT_B[:, ts(lt, 128)], gBt[p][:],
                                 start=False, stop=True)
                nc.vector.tensor_add(out_s[:, ts(lt, 512)], ob[:],
                                     e_s[:, ts(2 * p + lt, 512)])
            # store per o: [q, lt, n] <- out_s cols (lt, o, n)
            for half in range(2):
                dst = out_d[2 * p + half].rearrange("(t q) n -> q t n", q=128)
                srcap = out_s[:].rearrange("q (t o n) -> q t o n",
                                           t=2, o=2)[:, :, half]
                eng = nc.gpsimd if p == 1 else nc.sync
                eng.dma_start(dst, srcap)

        out_bank(0)
        g_build(2)
        nc.scalar.dma_start(gBt[2][GB:GB + 1, :], pjf[4:6, 0:N])
        out_bank(1)
        out_bank(2)

    nc.compile()
    return nc


def _get_module(has_bias: bool):
    key = ("mod", has_bias)
    if key not in _cache:
        _cache[key] = _build_module(has_bias)
    return _cache[key]


def _host_pack(head_w, head_b, tail_w, tail_b, U_mh, size_emb, W, down_w,
               down_b):
    """Fold down_w into the constants; build per-o-half bf16/fp8 blobs."""
    from ml_dtypes import bfloat16, float8_e4m3
    f64 = np.float64
    d1 = D + 1
    Wh, Wt, Ws = W[:, :d1], W[:, d1:2 * d1], W[:, 2 * d1:]
    WhD = (down_w.astype(f64) @ Wh.astype(f64)).astype(np.float32)  # [OUT,D+1]
    WtD = (down_w.astype(f64) @ Wt.astype(f64)).astype(np.float32)
    WsD = (down_w.astype(f64) @ Ws.astype(f64)).astype(np.float32)  # [OUT,SZ]
    ct = (size_emb.astype(f64) @ WsD.T.astype(f64)).astype(np.float32)
    dw_r = down_w.reshape(OUT, NH, HD)
    Up = np.einsum('ohd,hdxy->ohxy', dw_r.astype(f64),
                   U_mh.astype(f64)).astype(np.float32)  # [OUT,NH,HD,HD]

    idx = np.arange(N)
    span = np.clip(idx[None, :] - idx[:, None], -N_POS // 2,
                   N_POS // 2 - 1) + N_POS // 2
    E = ct[span].transpose(2, 0, 1) + down_b[:, None, None]  # [OUT,N,N]

    has_bias = bool(np.any(head_b) or np.any(tail_b))

    # blob1 constant part: per chunk k cols [wrt(N) | tw_k | hw_k]
    b1t = np.zeros((128, NK * CHC), np.float32)
    twp = tail_w.T.reshape(NK, 128, D)
    hwp = head_w.T.reshape(NK, 128, D)
    for k in range(NK):
        b1t[:, k * CHC + N:k * CHC + N + D] = twp[k]
        b1t[:, k * CHC + N + D:(k + 1) * CHC] = hwp[k]
    b1t = b1t.astype(bfloat16)

    blobs_oh = []
    for oh in range(2):
        osl = slice(oh * OH, (oh + 1) * OH)
        b2 = np.zeros((128, C2), np.float32)
        UpS = Up[osl]
        bd_a = np.zeros((OH, GA, GA), np.float32)
        bd_b = np.zeros((OH, GB, GB), np.float32)
        for h in range(3):
            bd_a[:, h * HD:(h + 1) * HD, h * HD:(h + 1) * HD] = \
                UpS[:, h].transpose(0, 2, 1)
        for h in range(2):
            bd_b[:, h * HD:(h + 1) * HD, h * HD:(h + 1) * HD] = \
                UpS[:, 3 + h].transpose(0, 2, 1)
        b2[0:GA, 0:OH * GA] = bd_a.transpose(1, 0, 2).reshape(GA, OH * GA)
        c = OH * GA
        b2[0:GA, c:c + OH] = WhD[osl, 0:GA].T
        b2[0:GA, c + OH:c + 2 * OH] = WtD[osl, 0:GA].T
        if has_bias:
            b2[0:GA, c + 2 * OH] = head_b[0:GA]
            b2[0:GA, c + 2 * OH + 1] = tail_b[0:GA]
        c = RA_W
        b2[0:GB, c:c + OH * GB] = bd_b.transpose(1, 0, 2).reshape(GB, OH * GB)
        c = RA_W + OH * GB
        b2[0:GB + 1, c:c + OH] = np.concatenate(
            [WhD[osl, GA:D].T, WhD[osl, D:D + 1].T], axis=0)
        b2[0:GB + 1, c + OH:c + 2 * OH] = np.concatenate(
            [WtD[osl, GA:D].T, WtD[osl, D:D + 1].T], axis=0)
        if has_bias:
            b2[0:GB, c + 2 * OH] = head_b[GA:D]
            b2[0:GB, c + 2 * OH + 1] = tail_b[GA:D]
        # indicator rows: slab rows [6p : 6p+6] go to gB[p] rows 81..86;
        # within the slab only rows 2p (o_even, cols 0:N) and 2p+1 are set
        c = RA_W + RB_W
        for p in range(OH // 2):
            b2[6 * p + 2 * p, c:c + N] = 1.0
            b2[6 * p + 2 * p + 1, c + N:c + 512] = 1.0
        b2[0, C_ONES:C_ONES + N] = 1.0

        e_pack = np.zeros((128, OH * 512), np.float32)
        for p in range(OH // 2):
            for lt in range(2):
                o0 = oh * OH + 2 * p
                c0 = (2 * p + lt) * 512
                e_pack[:, c0:c0 + N] = E[o0, lt * 128:(lt + 1) * 128, :]
                e_pack[:, c0 + N:c0 + 512] = E[o0 + 1,
                                               lt * 128:(lt + 1) * 128, :]

        blobs_oh.append((b2.astype(bfloat16), e_pack.astype(float8_e4m3)))
    return b1t, blobs_oh, has_bias


def _ensure_axon():
    """If a host-side jax.config pinned the cpu platform (e.g. to run the
    reference), switch back to the axon/neuron backend for the device run."""
    import jax
    try:
        if any(getattr(d, 'platform', '') == 'axon' for d in jax.devices()):
            return
    except Exception:
        pass
    try:
        import jax.extend
        jax.config.update('jax_platforms', 'axon')
        jax.extend.backend.clear_backends()
    except Exception:
        pass


def _make_in_maps(word_reps, b1t, blobs_oh):
    from ml_dtypes import bfloat16
    wrt_b = []
    for b in range(B):
        wrt_b.append(word_reps[b].T.reshape(NK, 128, N).astype(bfloat16))
    in_maps = []
    for core in range(NCORES):
        b, oh = core // 2, core % 2
        b2, ep = blobs_oh[oh]
        b1 = b1t.copy()
        for k in range(NK):
            b1[:, k * CHC:k * CHC + N] = wrt_b[b][k]
        in_maps.append(dict(blob1=b1, blob2=b2, e_pack=ep))
    return in_maps


def kernel(word_reps, cls_embeding=None, pieces_index=None, loss_mask=None,
           head_w=None, head_b=None, tail_w=None, tail_b=None, U_mh=None,
           size_emb=None, W=None, down_w=None, down_b=None, **_unused):
    global LAST_RESULT
    from concourse import bass_utils

    word_reps = np.asarray(word_reps, np.float32)
    args = [np.asarray(a, np.float32) for a in
            (head_w, head_b, tail_w, tail_b, U_mh, size_emb, W, down_w,
             down_b)]
    b1t, blobs_oh, has_bias = _host_pack(*args)

    nc = _get_module(has_bias)

    in_maps = _make_in_maps(word_reps, b1t, blobs_oh)
    _ensure_axon()

    trace = bool(os.environ.get("KERNEL_TRACE"))
    res = bass_utils.run_bass_kernel_spmd(nc, in_maps, list(range(NCORES)),
                                          trace=trace)
    LAST_RESULT = res

    out = np.empty((B, OUT, N, N), np.float32)
    for core in range(NCORES):
        b, oh = core // 2, core % 2
        out[b, oh * OH:(oh + 1) * OH] = \
            np.asarray(res.results[core]["out"], np.float32)
    return out
